# revision 10
# baseline (speedup 1.0000x reference)
"""MultiHeadedAttention Trainium2 kernel.

Problem: B=2, S=4096, d_model=512, H=8 heads, dk=64.
  q/k/v proj -> scaled dot-product attention per head -> concat -> out proj.

Sharding: 8 cores = (batch b in {0,1}) x (head-pair hp in {0..3}).
Each core computes, for its batch and its 2 heads:
  QpT/KpT/VpT = (x @ W[:, hp*128:hp*128+128] + b).T          [128=2*dk, 4096]
  S^T[k, q]   = Kp Qp^T (per head, fp32r matmuls, k-major)
  P^T         = exp(S^T / 8)  (ScalarE, reads PSUM, writes SBUF)
  ctxT, Z     = [Vp | 1].T @ P^T  (ones column gives softmax denominator)
  ctxT       /= Z  (reciprocal + partition_broadcast + multiply)
  outT_partial= Wo[hp*128:...]^T-chunks @ ctx   -> [512, 4096] (transposed)
Host: out[b] = sum_hp(outT_partial).T + bo.

Inputs are fed pre-transposed ([d_model, S], c-major) so projection matmuls
contract over the partition dimension with no on-device transposes of x.
"""

import numpy as np

import concourse.bass as bass
import concourse.bacc as bacc
import concourse.mybir as mybir
import concourse.tile as tile
from concourse.bass_utils import run_bass_kernel_spmd
from concourse.masks import make_identity

F32 = mybir.dt.float32
F32R = mybir.dt.float32r
EXP = mybir.ActivationFunctionType.Exp

B = 2
S = 4096
D = 512           # d_model
H = 8
DK = 64
HP = 4            # head pairs per batch
DL = 128          # local channels per core (2 heads)
CJ = 4            # contraction chunks of 128 over d_model
QT = S // 512     # 8 q-tiles of 512
KB = S // 128     # 32 k-blocks of 128
SCALE = 1.0 / np.sqrt(DK).item()  # 1/8

TRACE = False
LAST_RESULTS = None

_prog_cache = {}


def _emit(nc):
    xqT = nc.declare_dram_parameter("xqT", [D, S], F32, isOutput=False)
    xkT = nc.declare_dram_parameter("xkT", [D, S], F32, isOutput=False)
    xvT = nc.declare_dram_parameter("xvT", [D, S], F32, isOutput=False)
    wq = nc.declare_dram_parameter("wq", [D, DL], F32, isOutput=False)
    wk = nc.declare_dram_parameter("wk", [D, DL], F32, isOutput=False)
    wv = nc.declare_dram_parameter("wv", [D, DL], F32, isOutput=False)
    bq = nc.declare_dram_parameter("bq", [DL, 1], F32, isOutput=False)
    bk = nc.declare_dram_parameter("bk", [DL, 1], F32, isOutput=False)
    bv = nc.declare_dram_parameter("bv", [DL, 1], F32, isOutput=False)
    wo = nc.declare_dram_parameter("wo", [DL, D], F32, isOutput=False)
    outT = nc.declare_dram_parameter("outT", [D, S], F32, isOutput=True)

    with tile.TileContext(nc) as tc:
        with (
            nc.allow_low_precision(reason="fp32r (fp22-mantissa) matmul inputs"),
            tc.tile_pool(name="singles", bufs=1) as singles,
            tc.tile_pool(name="xin", bufs=4) as xin,
            tc.tile_pool(name="proj", bufs=1) as proj,
            tc.tile_pool(name="pt", bufs=4) as ptpool,
            tc.tile_pool(name="ctx", bufs=2) as ctxpool,
            tc.tile_pool(name="outp", bufs=3) as outpool,
            tc.tile_pool(name="norm", bufs=4) as normpool,
            tc.tile_pool(name="mm512", bufs=2, space="PSUM") as mmps,
            tc.tile_pool(name="sps", bufs=2, space="PSUM") as spsum,
            tc.tile_pool(name="cps", bufs=2, space="PSUM") as cpsum,
        ):
            # --- constants / weights ---
            ident = singles.tile([128, 128], F32, tag="ident")
            make_identity(nc, ident)
            ones_stage = singles.tile([128, 64], F32, tag="ones_stage")
            nc.vector.memset(ones_stage, 1.0)
            ones64 = singles.tile([1, 64], F32R, tag="ones64")
            nc.vector.tensor_copy(ones64, ones_stage[0:1, :])

            w_sb = {}
            for name, w in (("wq", wq), ("wk", wk), ("wv", wv)):
                t = singles.tile([128, CJ, DL], F32R, tag=name)
                nc.sync.dma_start(
                    out=t,
                    in_=w[:].rearrange("(j p) d -> p j d", p=128).bitcast(F32R),
                )
                w_sb[name] = t
            wo_sb = singles.tile([DL, D], F32R, tag="wo")
            nc.sync.dma_start(out=wo_sb, in_=wo[:].bitcast(F32R))
            b_sb = {}
            for name, bias in (("bq", bq), ("bk", bk), ("bv", bv)):
                t = singles.tile([DL, 1], F32, tag=name)
                nc.sync.dma_start(out=t, in_=bias[:])
                b_sb[name] = t

            # --- projections: dst = (x @ W + b).T, channel-major [128, S] ---
            qpT = proj.tile([DL, S], F32R, tag="qpT")
            kpT = proj.tile([DL, S], F32R, tag="kpT")
            vpT = proj.tile([DL, S], F32, tag="vpT")

            def project(xT, wname, dst):
                xTr = xT[:].rearrange("(j p) s -> p j s", p=128)
                for st in range(QT):
                    xt = xin.tile([128, CJ, 512], F32R, tag="xin")
                    nc.sync.dma_start(
                        out=xt,
                        in_=xTr[:, :, st * 512 : (st + 1) * 512].bitcast(F32R),
                    )
                    ps = mmps.tile([128, 512], F32, tag="mm512")
                    for cj in range(CJ):
                        nc.tensor.matmul(
                            ps,
                            lhsT=w_sb[wname][:, cj, :],
                            rhs=xt[:, cj, :],
                            start=(cj == 0),
                            stop=(cj == CJ - 1),
                        )
                    nc.vector.tensor_scalar_add(
                        dst[:, st * 512 : (st + 1) * 512], ps, b_sb["b" + wname[1]]
                    )

            project(xqT, "wq", qpT)
            project(xkT, "wk", kpT)
            project(xvT, "wv", vpT)

            # --- Vp: s-major [128 k, KB, 130] with ones cols at 64 and 129 ---
            vp = proj.tile([128, KB, 130], F32R, tag="vp")
            nc.vector.tensor_copy(vp[:, :, 64], ones_stage[:, 0:KB])
            nc.vector.tensor_copy(vp[:, :, 129], ones_stage[:, 0:KB])
            for kb in range(KB):
                tp = mmps.tile([128, 512], F32, tag="mm512")
                nc.tensor.transpose(
                    tp[:, 0:128], vpT[:, kb * 128 : (kb + 1) * 128], ident
                )
                nc.vector.tensor_copy(vp[:, kb, 0:64], tp[:, 0:64])
                nc.vector.tensor_copy(vp[:, kb, 65:129], tp[:, 64:128])

            # --- attention + output projection, per q-tile of 512 ---
            for qt in range(QT):
                qs = slice(qt * 512, (qt + 1) * 512)
                cps0 = cpsum.tile([128, 512], F32, tag="cps")
                cps1 = cpsum.tile([128, 512], F32, tag="cps")
                cps_h = (cps0, cps1)
                for kb in range(KB):
                    sp = spsum.tile([128, 1024], F32, tag="sps")
                    for h in (0, 1):
                        nc.tensor.matmul(
                            sp[:, h * 512 : (h + 1) * 512],
                            lhsT=kpT[
                                h * 64 : (h + 1) * 64, kb * 128 : (kb + 1) * 128
                            ],
                            rhs=qpT[h * 64 : (h + 1) * 64, qs],
                            start=True,
                            stop=True,
                        )
                    pt = ptpool.tile([128, 1024], F32R, tag="pt")
                    nc.scalar.activation(pt, sp, EXP, scale=SCALE)
                    for h in (0, 1):
                        nc.tensor.matmul(
                            cps_h[h][0:65, :],
                            lhsT=vp[:, kb, 65 * h : 65 * h + 65],
                            rhs=pt[:, h * 512 : (h + 1) * 512],
                            start=(kb == 0),
                            stop=(kb == KB - 1),
                        )
                # normalize: ctx[d, q] = ctx_unnorm[d, q] / Z[q]
                ctx = ctxpool.tile([DL, 512], F32R, tag="ctx")
                for h in (0, 1):
                    rec = normpool.tile([1, 512], F32R, tag="rec")
                    nc.vector.reciprocal(rec, cps_h[h][64:65, :])
                    # broadcast rec across 64 partitions via PE outer product
                    bc_ps = mmps.tile([128, 512], F32, tag="mm512")
                    nc.tensor.matmul(
                        bc_ps[0:64, :],
                        lhsT=ones64,
                        rhs=rec,
                        start=True,
                        stop=True,
                    )
                    bc = normpool.tile([64, 512], F32, tag="bc")
                    nc.vector.tensor_copy(bc, bc_ps[0:64, :])
                    nc.vector.tensor_mul(
                        ctx[h * 64 : (h + 1) * 64, :], cps_h[h][0:64, :], bc
                    )
                # out^T[dm, q] partial = (ctx^T @ Wo_slice)^T = Wo_chunk.T @ ctx
                for j in range(CJ):
                    ops = mmps.tile([128, 512], F32, tag="mm512")
                    nc.tensor.matmul(
                        ops,
                        lhsT=wo_sb[:, j * 128 : (j + 1) * 128],
                        rhs=ctx,
                        start=True,
                        stop=True,
                    )
                    ot = outpool.tile([128, 512], F32, tag="out")
                    nc.vector.tensor_copy(ot, ops)
                    nc.sync.dma_start(
                        out=outT[j * 128 : (j + 1) * 128, qs], in_=ot
                    )
    return nc


def _build():
    if "nc" not in _prog_cache:
        nc = bacc.Bacc()
        _emit(nc)
        nc.compile()
        _prog_cache["nc"] = nc
    return _prog_cache["nc"]


def _make_in_maps(query, key, value, Wq, bq, Wk, bk, Wv, bv, Wo):
    in_maps = []
    for b in range(B):
        xqT = np.ascontiguousarray(query[b].T)
        xkT = np.ascontiguousarray(key[b].T)
        xvT = np.ascontiguousarray(value[b].T)
        for hp in range(HP):
            cs = slice(hp * DL, (hp + 1) * DL)
            in_maps.append(
                {
                    "xqT": xqT,
                    "xkT": xkT,
                    "xvT": xvT,
                    "wq": np.ascontiguousarray(Wq[:, cs]),
                    "wk": np.ascontiguousarray(Wk[:, cs]),
                    "wv": np.ascontiguousarray(Wv[:, cs]),
                    "bq": np.ascontiguousarray(bq[cs].reshape(DL, 1)),
                    "bk": np.ascontiguousarray(bk[cs].reshape(DL, 1)),
                    "bv": np.ascontiguousarray(bv[cs].reshape(DL, 1)),
                    "wo": np.ascontiguousarray(Wo[cs, :]),
                }
            )
    return in_maps


def kernel(query, key, value, Wq, bq, Wk, bk, Wv, bv, Wo, bo):
    global LAST_RESULTS
    query = np.asarray(query, dtype=np.float32)
    key = np.asarray(key, dtype=np.float32)
    value = np.asarray(value, dtype=np.float32)
    Wq = np.asarray(Wq, dtype=np.float32)
    Wk = np.asarray(Wk, dtype=np.float32)
    Wv = np.asarray(Wv, dtype=np.float32)
    Wo = np.asarray(Wo, dtype=np.float32)
    bq = np.asarray(bq, dtype=np.float32)
    bk = np.asarray(bk, dtype=np.float32)
    bv = np.asarray(bv, dtype=np.float32)
    bo = np.asarray(bo, dtype=np.float32)

    nc = _build()
    in_maps = _make_in_maps(query, key, value, Wq, bq, Wk, bk, Wv, bv, Wo)

    res = run_bass_kernel_spmd(nc, in_maps, list(range(B * HP)), trace=TRACE)
    LAST_RESULTS = res

    out = np.empty((B, S, D), dtype=np.float32)
    for b in range(B):
        acc = res.results[b * HP]["outT"].astype(np.float32)
        for hp in range(1, HP):
            acc = acc + res.results[b * HP + hp]["outT"]
        out[b] = acc.T + bo
    return out


# revision 36
# speedup vs baseline: 37.2007x; 37.2007x over previous
"""MultiHeadedAttention Trainium2 kernel.

Problem: B=2, S=4096, d_model=512, H=8 heads, dk=64.
  q/k/v proj -> scaled dot-product attention per head -> concat -> out proj.

Sharding: 8 cores = (batch b in {0,1}) x (head-pair hp in {0..3}).
Each core computes, for its batch and its 2 heads:
  QpT/KpT/VpT = (x @ W[:, hp*128:hp*128+128] + b).T          [128=2*dk, 4096]
  S^T[k, q]   = Kp Qp^T (per head, fp32r matmuls, k-major)
  P^T         = exp(S^T / 8)  (ScalarE, reads PSUM, writes SBUF)
  ctxT, Z     = [Vp | 1].T @ P^T  (ones column gives softmax denominator)
  ctxT       /= Z  (reciprocal + partition_broadcast + multiply)
  outT_partial= Wo[hp*128:...]^T-chunks @ ctx   -> [512, 4096] (transposed)
Host: out[b] = sum_hp(outT_partial).T + bo.

Inputs are fed pre-transposed ([d_model, S], c-major) so projection matmuls
contract over the partition dimension with no on-device transposes of x.
"""

import numpy as np

import concourse.bass as bass
import concourse.bacc as bacc
import concourse.mybir as mybir
import concourse.tile as tile
from concourse.bass_utils import run_bass_kernel_spmd
from concourse.masks import make_identity

F32 = mybir.dt.float32
F32R = mybir.dt.float32r
EXP = mybir.ActivationFunctionType.Exp

B = 2
S = 4096
D = 512           # d_model
H = 8
DK = 64
HP = 4            # head pairs per batch
DL = 128          # local channels per core (2 heads)
CJ = 4            # contraction chunks of 128 over d_model
QT = S // 512     # 8 q-tiles of 512
KB = S // 128     # 32 k-blocks of 128
SCALE = 1.0 / np.sqrt(DK).item()  # 1/8

TRACE = False
LAST_RESULTS = None

_prog_cache = {}


def _emit(nc, reps=1):
    xqT = nc.declare_dram_parameter("xqT", [D, S], F32, isOutput=False)
    xkT = nc.declare_dram_parameter("xkT", [D, S], F32, isOutput=False)
    xvT = nc.declare_dram_parameter("xvT", [D, S], F32, isOutput=False)
    wq = nc.declare_dram_parameter("wq", [D, DL], F32, isOutput=False)
    wk = nc.declare_dram_parameter("wk", [D, DL], F32, isOutput=False)
    wv = nc.declare_dram_parameter("wv", [D, DL], F32, isOutput=False)
    bq = nc.declare_dram_parameter("bq", [DL, 1], F32, isOutput=False)
    bk = nc.declare_dram_parameter("bk", [DL, 1], F32, isOutput=False)
    bv = nc.declare_dram_parameter("bv", [DL, 1], F32, isOutput=False)
    wo = nc.declare_dram_parameter("wo", [DL, D], F32, isOutput=False)
    outT = nc.declare_dram_parameter("outT", [D, S], F32, isOutput=True)

    with tile.TileContext(nc) as tc:
        with (
            nc.allow_low_precision(reason="fp32r (fp22-mantissa) matmul inputs"),
            tc.tile_pool(name="singles", bufs=1) as singles,
            tc.tile_pool(name="xin", bufs=4) as xin,
            tc.tile_pool(name="proj", bufs=1) as proj,
            tc.tile_pool(name="pt", bufs=6) as ptpool,
            tc.tile_pool(name="ctx", bufs=2) as ctxpool,
            tc.tile_pool(name="outp", bufs=4) as outpool,
            tc.tile_pool(name="norm", bufs=6) as normpool,
            tc.tile_pool(name="mm512", bufs=2, space="PSUM") as mmps,
            tc.tile_pool(name="sps", bufs=2, space="PSUM") as spsum,
            tc.tile_pool(name="cps", bufs=2, space="PSUM") as cpsum,
        ):
            # --- constants / weights ---
            ident = singles.tile([128, 128], F32, tag="ident")
            make_identity(nc, ident)
            ones_stage = singles.tile([128, 64], F32, tag="ones_stage")
            nc.vector.memset(ones_stage, 1.0)
            ones64 = singles.tile([1, 64], F32R, tag="ones64")
            nc.vector.tensor_copy(ones64, ones_stage[0:1, :])
            warm = singles.tile([1, 8], F32, tag="warm")
            nc.scalar.activation(warm, ones_stage[0:1, 0:8], EXP, scale=1.0)

            w_sb = {}
            b_sb = {}

            def load_w(name, w, bias):
                t = singles.tile([128, CJ, DL], F32R, tag=name)
                nc.sync.dma_start(
                    out=t,
                    in_=w[:].rearrange("(j p) d -> p j d", p=128).bitcast(F32R),
                )
                w_sb[name] = t
                bt = singles.tile([DL, 1], F32, tag="b" + name[1])
                nc.sync.dma_start(out=bt, in_=bias[:])
                b_sb["b" + name[1]] = bt

            # --- projections: dst = (x @ W + b).T, channel-major [128, S] ---
            qpT = proj.tile([DL, S], F32R, tag="qpT")
            kpT = proj.tile([DL, S], F32R, tag="kpT")
            vpT = proj.tile([DL, S], F32, tag="vpT")

            vp = proj.tile([128, KB, 130], F32R, tag="vp")
            nc.vector.tensor_copy(vp[:, :, 64], ones_stage[:, 0:KB])
            nc.vector.tensor_copy(vp[:, :, 129], ones_stage[:, 0:KB])

            def project_cols(xT, wname, dst, c0, w):
                """Columns [c0, c0+w) of dst = (x @ W + b).T"""
                xTr = xT[:].rearrange("(j p) s -> p j s", p=128)
                xt = xin.tile([128, CJ, 512], F32R, tag="xin")
                nc.sync.dma_start(
                    out=xt[:, :, 0:w],
                    in_=xTr[:, :, c0 : c0 + w].bitcast(F32R),
                )
                ps = mmps.tile([128, 512], F32, tag="mm512")
                for cj in range(CJ):
                    nc.tensor.matmul(
                        ps[:, 0:w],
                        lhsT=w_sb[wname][:, cj, :],
                        rhs=xt[:, cj, 0:w],
                        start=(cj == 0),
                        stop=(cj == CJ - 1),
                    )
                nc.vector.tensor_scalar_add(
                    dst[:, c0 : c0 + w], ps[:, 0:w], b_sb["b" + wname[1]]
                )

            def project_st(xT, wname, dst, st):
                project_cols(xT, wname, dst, st * 512, 512)

            def v_transpose_st(st):
                """Vp s-major blocks for the 4 k-blocks of one s-tile."""
                for kb in range(st * 4, st * 4 + 4):
                    tp = mmps.tile([128, 512], F32, tag="mm512")
                    nc.tensor.transpose(
                        tp[:, 0:128], vpT[:, kb * 128 : (kb + 1) * 128], ident
                    )
                    nc.vector.tensor_copy(vp[:, kb, 0:64], tp[:, 0:64])
                    nc.vector.tensor_copy(vp[:, kb, 65:129], tp[:, 64:128])

            # Streaming order chosen so the attention frontier unlocks ASAP:
            # q-tile 0 first, then K/V interleaved per s-tile (each s-tile
            # unlocks 4 k-blocks for scores+ctx), remaining Q tiles last.
            load_w("wq", wq, bq)
            project_st(xqT, "wq", qpT, 0)
            load_w("wk", wk, bk)
            load_w("wv", wv, bv)
            wo_sb = singles.tile([DL, D], F32R, tag="wo")
            nc.sync.dma_start(out=wo_sb, in_=wo[:].bitcast(F32R))
            for st in range(QT):
                project_st(xkT, "wk", kpT, st)
                project_st(xvT, "wv", vpT, st)
                v_transpose_st(st)
            for st in range(1, QT):
                project_st(xqT, "wq", qpT, st)

            # --- attention + output projection, per q-tile of 512 ---
            # Epilogue work (normalize + Wo projection) for q-tile qt is
            # emitted piecewise during q-tile qt+1's kb loop so the PE queue
            # never stalls ACT at the boundary.
            state = {}

            def normalize_h(qt, cps_h, h):
                if h == 0:
                    state["ctx"] = ctxpool.tile([DL, 512], F32R, tag="ctx", name="ctx")
                rec = normpool.tile([1, 512], F32R, tag="rec")
                nc.vector.reciprocal(rec, cps_h[h][64:65, :])
                bc = normpool.tile([64, 512], F32, tag="bc")
                nc.gpsimd.partition_broadcast(bc, rec.bitcast(F32))
                nc.vector.tensor_mul(
                    state["ctx"][h * 64 : (h + 1) * 64, :], cps_h[h][0:64, :], bc
                )

            def oproj_j(qt, j):
                qs = slice(qt * 512, (qt + 1) * 512)
                ops = mmps.tile([128, 512], F32, tag="mm512")
                nc.tensor.matmul(
                    ops,
                    lhsT=wo_sb[:, j * 128 : (j + 1) * 128],
                    rhs=state["ctx"],
                    start=True,
                    stop=True,
                )
                ot = outpool.tile([128, 512], F32, tag="out")
                nc.vector.tensor_copy(ot, ops)
                nc.sync.dma_start(out=outT[j * 128 : (j + 1) * 128, qs], in_=ot)

            def epilogue_step(step, qt, cps_h):
                if step == 1:
                    normalize_h(qt, cps_h, 0)
                elif step == 2:
                    normalize_h(qt, cps_h, 1)
                elif 3 <= step <= 6:
                    oproj_j(qt, step - 3)

            def scores_exp(qt, kb):
                qs = slice(qt * 512, (qt + 1) * 512)
                sp = spsum.tile([128, 1024], F32, tag="sps")
                for h in (0, 1):
                    nc.tensor.matmul(
                        sp[:, h * 512 : (h + 1) * 512],
                        lhsT=kpT[h * 64 : (h + 1) * 64, kb * 128 : (kb + 1) * 128],
                        rhs=qpT[h * 64 : (h + 1) * 64, qs],
                        start=True,
                        stop=True,
                    )
                pt = ptpool.tile([128, 1024], F32R, tag="pt")
                nc.scalar.activation(pt, sp, EXP, scale=SCALE)
                return pt

            def ctx_mm(cps_h, kb, pt):
                for h in (0, 1):
                    nc.tensor.matmul(
                        cps_h[h][0:65, :],
                        lhsT=vp[:, kb, 65 * h : 65 * h + 65],
                        rhs=pt[:, h * 512 : (h + 1) * 512],
                        start=(kb == 0),
                        stop=(kb == KB - 1),
                    )

            # Software pipeline: ctx(kb-1) is emitted after scores/exp(kb) so
            # the in-order PE queue never makes ACT wait a full ctx+scores hop.
            pending = None  # (qt, cps_h) awaiting epilogue
            for qt in [q for _ in range(reps) for q in range(QT)]:
                cps0 = cpsum.tile([128, 512], F32, tag="cps")
                cps1 = cpsum.tile([128, 512], F32, tag="cps")
                cps_h = (cps0, cps1)
                pts = {}
                for kb in range(KB):
                    pts[kb] = scores_exp(qt, kb)
                    if kb >= 2:
                        ctx_mm(cps_h, kb - 2, pts.pop(kb - 2))
                    if pending is not None:
                        epilogue_step(kb, *pending)
                ctx_mm(cps_h, KB - 2, pts.pop(KB - 2))
                ctx_mm(cps_h, KB - 1, pts.pop(KB - 1))
                pending = (qt, cps_h)
            for step in range(1, 7):
                epilogue_step(step, *pending)
    return nc


def _build(reps=1):
    if reps not in _prog_cache:
        nc = bacc.Bacc()
        _emit(nc, reps)
        nc.compile()
        _prog_cache[reps] = nc
    return _prog_cache[reps]


def _make_in_maps(query, key, value, Wq, bq, Wk, bk, Wv, bv, Wo):
    in_maps = []
    for b in range(B):
        xqT = np.ascontiguousarray(query[b].T)
        xkT = np.ascontiguousarray(key[b].T)
        xvT = np.ascontiguousarray(value[b].T)
        for hp in range(HP):
            cs = slice(hp * DL, (hp + 1) * DL)
            in_maps.append(
                {
                    "xqT": xqT,
                    "xkT": xkT,
                    "xvT": xvT,
                    "wq": np.ascontiguousarray(Wq[:, cs]),
                    "wk": np.ascontiguousarray(Wk[:, cs]),
                    "wv": np.ascontiguousarray(Wv[:, cs]),
                    "bq": np.ascontiguousarray(bq[cs].reshape(DL, 1)),
                    "bk": np.ascontiguousarray(bk[cs].reshape(DL, 1)),
                    "bv": np.ascontiguousarray(bv[cs].reshape(DL, 1)),
                    "wo": np.ascontiguousarray(Wo[cs, :]),
                }
            )
    return in_maps


def kernel(query, key, value, Wq, bq, Wk, bk, Wv, bv, Wo, bo):
    global LAST_RESULTS
    query = np.asarray(query, dtype=np.float32)
    key = np.asarray(key, dtype=np.float32)
    value = np.asarray(value, dtype=np.float32)
    Wq = np.asarray(Wq, dtype=np.float32)
    Wk = np.asarray(Wk, dtype=np.float32)
    Wv = np.asarray(Wv, dtype=np.float32)
    Wo = np.asarray(Wo, dtype=np.float32)
    bq = np.asarray(bq, dtype=np.float32)
    bk = np.asarray(bk, dtype=np.float32)
    bv = np.asarray(bv, dtype=np.float32)
    bo = np.asarray(bo, dtype=np.float32)

    nc = _build()
    in_maps = _make_in_maps(query, key, value, Wq, bq, Wk, bk, Wv, bv, Wo)

    res = run_bass_kernel_spmd(nc, in_maps, list(range(B * HP)), trace=TRACE)
    LAST_RESULTS = res

    out = np.empty((B, S, D), dtype=np.float32)
    for b in range(B):
        acc = res.results[b * HP]["outT"].astype(np.float32)
        for hp in range(1, HP):
            acc = acc + res.results[b * HP + hp]["outT"]
        out[b] = acc.T + bo
    return out
